# revision 31
# baseline (speedup 1.0000x reference)
"""Cross-attention layer (B=2, T=2048, C=3072, D=1024, 16 heads x 64) on 8 TRN2 cores.

Sharding: batch (2) x head-groups (4). Core i handles batch b=i//4 and the 4
heads [4*(i%4), 4*(i%4)+4). Q/K/V projections are column-sharded by head,
the output projection is row-sharded, so each core returns a partial [T, D]
output (bf16); the host sums the 4 partials per batch element and adds bo.

On-core dataflow (bf16 matmuls, fp32 PSUM accum), ~340us/core measured:
  DMA (sync HWDGE queue, priority order): one jumbo descriptor for all
  QKV weights, then x chunks, then ctx chunks; small constants ride the
  scalar HWDGE queue concurrently.
  phase A: Q projection (k-outer over x chunks as they land), then two
  V passes that share one 8-slot PSUM ring with Q and with four early
  K-projection blocks (kT[0] cq0-3) that ride the ctx-DMA-paced PE idle;
  V output is PE-transposed into v[c, (ci,h)*65] (64 v cols + a ones col
  per head from a memset).
  attention runs per head-PAIR (2p, 2p+1), per T-half tb, 24 c-chunks:
  per ci and head: QK (row-tiled at PE position (64*h2, 0), AABB order so
  h0's matmuls are gated only by the early exp slot release), exp on ACT
  (scale=1/8 fused), then the previous ci's PV for that head fills the PE
  under the ACT slot. PV accumulates uo_ps[65, t] (row 64 = softmax
  denominator via the ones column — a free 65th output partition).
  The steady-state ci loop is ACT-bound (2 x 1147ns exp per ci); the
  remaining K blocks (pairs 1-2), denominators/normalization, and the
  first half of the output projection (pairs 3-4) run as just-in-time
  fillers. Every filler allocates an EVEN number of tag-"qk" PSUM tiles
  so the 2-slot ring keeps qk_h0 on the early-released slot.
  normalize: denominators broadcast via mask-matmul, reciprocal+mul on DVE
  out[t, D] = sum_p uo_p^T-slice @ WoT_shard    (bf16, accumulated in PSUM)
"""
from collections import deque

import numpy as np
import concourse.bass as bass  # noqa: F401  (engine types re-exported via nc)
import concourse.mybir as mybir
import concourse.tile as tile
from concourse import bacc
from concourse.bass import ds, ts
from concourse.bass_utils import run_bass_kernel_spmd
import ml_dtypes

_bf16 = ml_dtypes.bfloat16

B, T, C, D = 2, 2048, 3072, 1024
NH, DH = 16, 64
NCORES = 8
HL = 4                # heads per core
DL = HL * DH          # 256 local projection dims
DHP = DH + 1          # 65: per-head v columns incl. ones column

F32 = mybir.dt.float32
BF16 = mybir.dt.bfloat16
AF = mybir.ActivationFunctionType

KC = D // 128         # 8 contraction chunks for projections
CC = C // 128         # 24 key tiles
TT = T // 128         # 16 query tiles


def _emit(nc, tc, io):
    xT, ctxT, wqkvT, woT, bqkv, msk, iden, out = io

    with (
        tc.sbuf_pool(name="persist", bufs=1) as pp,
        tc.sbuf_pool(name="wqkv", bufs=1) as wp,
        tc.sbuf_pool(name="stream", bufs=1) as sp,
    ):
        qT = [pp.tile([128, T], BF16, name=f"qT{p}") for p in range(2)]
        kT = [pp.tile([128, C], BF16, name=f"kT{p}") for p in range(2)]
        v = pp.tile([128, CC * HL * DHP], BF16, name="v")
        wo = [pp.tile([128, D], BF16, name=f"wo{p}") for p in range(2)]
        msk_sb = pp.tile([65, 128], BF16, name="msk_sb")
        iden_sb = pp.tile([128, 128], BF16, name="iden_sb")
        b_sb = [pp.tile([128, 3], F32, name=f"b{p}") for p in range(2)]
        # ones columns for the softmax denominators (65th v column per head):
        # rides the PV matmul as a 65th output partition at zero stream cost.
        ones_view = v.rearrange("a (i c) -> a i c", c=DHP)[:, :, DH:DHP]
        nc.gpsimd.memset(ones_view, 1.0)

        # All weight chunks ride ONE jumbo DMA descriptor into a single
        # [128, KC*768] tile (k-chunk k at free offset k*768): one trigger
        # instead of 8 on the head-critical sync queue.
        w_all = wp.tile([128, KC * 3 * DL], BF16, name="w_all")
        w_sb = [w_all[:, ds(k * 3 * DL, 3 * DL)] for k in range(KC)]
        xs = [sp.tile([128, T], BF16, tag=f"x{k}", name=f"xs{k}") for k in range(KC)]
        cs = [sp.tile([128, C], BF16, tag=f"c{k}", name=f"cs{k}") for k in range(KC)]

        # Priority order on the sync HWDGE queue: w (one jumbo trigger) then
        # x then ctx — everything the projections block on, in need-order so
        # transfers don't compete with each other for HBM bandwidth. The
        # scalar HWDGE queue carries only the small constants concurrently.
        nc.sync.dma_start(
            out=w_all.rearrange("p (k c) -> p k c", k=KC),
            in_=wqkvT.rearrange("(k p) c -> p k c", k=KC),
        )
        for k in range(KC):
            nc.sync.dma_start(out=xs[k], in_=xT[ts(k, 128), :])
        for k in range(KC):
            nc.sync.dma_start(out=cs[k], in_=ctxT[ts(k, 128), :])
        nc.scalar.dma_start(out=iden_sb, in_=iden)
        for p in range(2):
            nc.scalar.dma_start(out=b_sb[p], in_=bqkv[ts(p, 128), :])
        nc.scalar.dma_start(out=msk_sb, in_=msk)
        for p in range(2):
            nc.scalar.dma_start(out=wo[p], in_=woT[ts(p, 128), :])

        bq_sb = [b_sb[p][:, 0:1] for p in range(2)]
        bk_sb = [b_sb[p][:, 1:2] for p in range(2)]
        bv_sb = [b_sb[p][:, 2:3] for p in range(2)]

        wq = [w_sb[k][:, 0:DL] for k in range(KC)]
        wk = [w_sb[k][:, DL : 2 * DL] for k in range(KC)]
        wv = [w_sb[k][:, 2 * DL : 3 * DL] for k in range(KC)]

        # ---------------- Phase A: Q, V, and half of K ----------------
        # One 8-slot PSUM ring ("proj") serves Q, V, early-K, and the
        # transpose staging: V's k-outer groups evict Q's as each Q group
        # drains, and the two K blocks ride the DMA-paced window of each
        # V pass (the PE idles ~0.5us per k-chunk waiting for ctx DMA
        # otherwise). kT[0] cq0-3 is therefore ready before attention
        # starts and pair 1 needs no pre-fillers.
        with (
            tc.sbuf_pool(name="vstage", bufs=6) as vp,
            tc.psum_pool(name="papool", bufs=8) as pa,
        ):
            qgr = [
                pa.tile([128, 512], F32, tag="proj", name=f"qp{p}{tq}")
                for p in range(2)
                for tq in range(4)
            ]
            for k in range(KC):
                for i in range(8):
                    p, tq = i // 4, i % 4
                    nc.tensor.matmul(
                        qgr[i],
                        lhsT=wq[k][:, ts(p, 128)],
                        rhs=xs[k][:, ts(tq, 512)],
                        start=(k == 0),
                        stop=(k == KC - 1),
                    )
            for i in range(8):
                p, tq = i // 4, i % 4
                nc.vector.tensor_scalar_add(
                    out=qT[p][:, ts(tq, 512)], in0=qgr[i], scalar1=bq_sb[p]
                )

            for p in range(2):
                vgr = [
                    pa.tile([128, 512], F32, tag="proj", name=f"vp{p}{cq}")
                    for cq in range(6)
                ]
                kgr = [
                    pa.tile([128, 512], F32, tag="proj", name=f"kg{p}{j}")
                    for j in range(2)
                ]
                for k in range(KC):
                    for cq in range(6):
                        nc.tensor.matmul(
                            vgr[cq],
                            lhsT=wv[k][:, ts(p, 128)],
                            rhs=cs[k][:, ts(cq, 512)],
                            start=(k == 0),
                            stop=(k == KC - 1),
                        )
                    for j in range(2):
                        nc.tensor.matmul(
                            kgr[j],
                            lhsT=wk[k][:, ts(0, 128)],
                            rhs=cs[k][:, ts(2 * p + j, 512)],
                            start=(k == 0),
                            stop=(k == KC - 1),
                        )
                for j in range(2):
                    nc.vector.tensor_scalar_add(
                        out=kT[0][:, ts(2 * p + j, 512)], in0=kgr[j], scalar1=bk_sb[0]
                    )
                vsts = []
                for cq in range(6):
                    vst = vp.tile([128, 512], BF16, tag="vst", name=f"vs{p}{cq}")
                    nc.vector.tensor_scalar_add(out=vst, in0=vgr[cq], scalar1=bv_sb[p])
                    vsts.append(vst)
                for cq in range(6):
                    for cb in range(4):
                        ci = cq * 4 + cb
                        tp_ = pa.tile(
                            [128, 128], BF16, tag="proj", name=f"tr{ci}{p}"
                        )
                        nc.tensor.transpose(tp_, vsts[cq][:, ts(cb, 128)], iden_sb)
                        src_ = tp_.rearrange("a (h c) -> a h c", h=2)
                        dst = v[
                            :, ds(ci * HL * DHP + 2 * p * DHP, 2 * DHP)
                        ].rearrange("a (h c) -> a h c", h=2)[:, :, 0:DH]
                        nc.vector.tensor_copy(out=dst, in_=src_)

        # ---------------- Phases B + C (K-proj + out-proj interleaved) ----
        with (
            tc.sbuf_pool(name="uop", bufs=1) as up,
            tc.sbuf_pool(name="rsp", bufs=1) as rp,
            tc.sbuf_pool(name="obp", bufs=3) as ob,
            tc.sbuf_pool(name="expp", bufs=4) as ep,
            tc.sbuf_pool(name="rcp", bufs=2) as rc,
            tc.psum_pool(name="qkps", bufs=2) as qps,
            tc.psum_pool(name="uops", bufs=2) as ups,
        ):
            uo = [up.tile([128, T], BF16, name=f"uo{pr}") for pr in range(2)]
            rs = [rp.tile([65, T], BF16, name=f"rs{pr}") for pr in range(2)]
            for pr in range(2):
                nc.gpsimd.memset(rs[pr], 1.0)

            def kproj_block(pp_, cq):
                def emit():
                    ps = qps.tile([128, 512], F32, tag="qk", name=f"kp{pp_}{cq}")
                    for k in range(KC):
                        nc.tensor.matmul(
                            ps,
                            lhsT=wk[k][:, ts(pp_, 128)],
                            rhs=cs[k][:, ts(cq, 512)],
                            start=(k == 0),
                            stop=(k == KC - 1),
                        )
                    nc.vector.tensor_scalar_add(
                        out=kT[pp_][:, ts(cq, 512)], in0=ps, scalar1=bk_sb[pp_]
                    )

                return emit

            def kproj_half(pp_, cq):
                a = kproj_quarter(pp_, cq, 0)
                b = kproj_quarter(pp_, cq, 1)

                def emit():
                    a()
                    b()

                return emit

            def kproj_quarter(pp_, cq, qh):
                # 256-wide K-projection block: ~1.3us of PE per invocation so
                # a single filler stays within the per-ci PE headroom under
                # the ACT exp slot (full 512 blocks overshoot and stall ACT).
                def emit():
                    off = cq * 512 + qh * 256
                    ps = qps.tile([128, 256], F32, tag="qk", name=f"kq{pp_}{cq}{qh}")
                    for k in range(KC):
                        nc.tensor.matmul(
                            ps,
                            lhsT=wk[k][:, ts(pp_, 128)],
                            rhs=cs[k][:, ds(off, 256)],
                            start=(k == 0),
                            stop=(k == KC - 1),
                        )
                    nc.vector.tensor_scalar_add(
                        out=kT[pp_][:, ds(off, 256)], in0=ps, scalar1=bk_sb[pp_]
                    )

                return emit

            def c_tile_half(tt, dc, osb_holder, on_act=False, pool=None):
                # half an out-projection t-tile (one 512-wide dc chunk);
                # split so a single filler invocation stays under the per-ci
                # PE headroom while ACT streams the exp.
                def emit():
                    if dc == 0:
                        osb_holder["t"] = ob.tile(
                            [128, D], BF16, tag="ob", name=f"ob{tt}"
                        )
                    osb = osb_holder["t"]
                    pl, tg = pool if pool is not None else (qps, "qk")
                    o_ps = pl.tile([128, 512], F32, tag=tg, name=f"o{tt}_{dc}")
                    for p in range(2):
                        nc.tensor.matmul(
                            o_ps,
                            lhsT=uo[p][:, ts(tt, 128)],
                            rhs=wo[p][:, ts(dc, 512)],
                            start=(p == 0),
                            stop=(p == 1),
                        )
                    if on_act and dc == 1:
                        nc.scalar.copy(out=osb[:, ts(dc, 512)], in_=o_ps)
                    else:
                        nc.vector.tensor_copy(out=osb[:, ts(dc, 512)], in_=o_ps)
                    if dc == 1:
                        nc.sync.dma_start(out=out[ts(tt, 128), :], in_=osb)

                return emit

            def c_tile(tt, on_act=False, pool=None):
                h = {}
                a = c_tile_half(tt, 0, h, on_act, pool)
                b = c_tile_half(tt, 1, h, on_act, pool)

                def emit():
                    a()
                    b()

                return emit

            def norm_full(pr, th):
                a = norm_half(pr, th, 0)
                b = norm_half(pr, th, 1)

                def emit():
                    a()
                    b()

                return emit

            def norm_half(pr, th, tq):
                # broadcast denominators -> [128, 512], recip, scale uo half
                def emit():
                    off = th * 1024 + tq * 512
                    bc = qps.tile([128, 512], F32, tag="qk", name=f"bc{pr}{th}{tq}")
                    nc.tensor.matmul(
                        bc,
                        lhsT=msk_sb,
                        rhs=rs[pr][:, ds(off, 512)],
                        start=True,
                        stop=True,
                    )
                    rcl = rc.tile([128, 512], F32, tag="rc", name=f"rcl{pr}{th}{tq}")
                    nc.vector.reciprocal_approx_fast(rcl, bc)
                    nc.vector.tensor_mul(
                        out=uo[pr][:, ds(off, 512)],
                        in0=uo[pr][:, ds(off, 512)],
                        in1=rcl,
                    )

                return emit

            def attn_pair(p, tb, pre, post):
                # heads (2p, 2p+1) together. QK is row-tiled at PE positions
                # (0,0)/(64,0), AABB per head so h0's matmuls are gated only
                # by exp(ci-1,h0)'s early slot release. The emission is
                # software-pipelined per head: QK_h(ci) and exp_h(ci) issue
                # first, then PV_h(ci-1) — whose ex operand became ready one
                # ACT slot ago — fills the PE while ACT streams the exps, and
                # QK_h1(ci) lands right as exp(ci-1,h1)'s slot frees. PV is
                # the 65-wide form: the ones column in v makes the softmax
                # denominator a free 65th output partition.
                uo_ps = [
                    ups.tile([65, 1024], F32, tag="uo", name=f"up{p}{tb}_{h2}")
                    for h2 in range(2)
                ]

                def pv_step(ci, h2, ex):
                    h = 2 * p + h2
                    vsl = v[:, ds((ci * HL + h) * DHP, DHP)]
                    for tq in range(2):
                        nc.tensor.matmul(
                            uo_ps[h2][:, ts(tq, 512)],
                            lhsT=vsl,
                            rhs=ex[:, ts(tq, 512)],
                            start=(ci == 0),
                            stop=(ci == CC - 1),
                        )

                prev = [None, None]
                for ci in range(CC):
                    for f in pre.get(ci, ()):
                        f()
                    exs = []
                    for h2 in range(2):
                        base = 64 * h2
                        qk = qps.tile(
                            [128, 1024], F32, tag="qk", name=f"qk{p}{tb}_{ci}_{h2}"
                        )
                        for tq in range(2):
                            nc.tensor.matmul(
                                qk[:, ts(tq, 512)],
                                lhsT=kT[p][ds(base, 64), ts(ci, 128)],
                                rhs=qT[p][ds(base, 64), ds(tb * 1024 + tq * 512, 512)],
                                start=True,
                                stop=True,
                                tile_position=(base, 0),
                            )
                        ex = ep.tile(
                            [128, 1024], BF16, tag="exp", name=f"ex{p}{tb}_{ci}_{h2}"
                        )
                        nc.scalar.activation(ex, qk, AF.Exp, scale=0.125)
                        exs.append(ex)
                        if prev[h2] is not None:
                            pv_step(ci - 1, h2, prev[h2])
                    prev = exs
                    for f in post.get(ci, ()):
                        f()
                # tail: last ci's PVs; drain runs on DVE at the boundary.
                for h2 in range(2):
                    pv_step(CC - 1, h2, prev[h2])

                def drain():
                    for h2 in range(2):
                        nc.vector.tensor_copy(
                            out=rs[p][ds(64 * h2, 1), ds(tb * 1024, 1024)],
                            in_=uo_ps[h2][64:65, :],
                        )
                    for h2 in range(2):
                        nc.vector.tensor_copy(
                            out=uo[p][ds(64 * h2, 64), ds(tb * 1024, 1024)],
                            in_=uo_ps[h2][0:64, :],
                        )

                return drain

            # Each pair's drain is emitted at the pair boundary (DVE, off
            # the ACT critical path); norms run as fillers inside the NEXT
            # pair. kT[0] cq0-3 came out of phase A; the remaining K blocks
            # feed pairs 1-2 just-in-time. Every filler allocates an EVEN
            # number of tag-"qk" PSUM tiles so the 2-slot ring's parity is
            # preserved (an odd filler would land the early-gated qk_h0 on
            # the late-released slot and stall the exp stream).
            # pair 1 (p0,tb0)
            dr = attn_pair(
                0,
                0,
                {},
                {
                    3: [kproj_half(0, 4)],
                    6: [kproj_half(0, 5)],
                    9: [kproj_half(1, 0)],
                    13: [kproj_half(1, 1)],
                    17: [kproj_half(1, 2)],
                    21: [kproj_half(1, 3)],
                },
            )
            dr()
            # pair 2 (p1,tb0)
            dr = attn_pair(
                1,
                0,
                {},
                {
                    3: [kproj_half(1, 4)],
                    7: [kproj_half(1, 5)],
                    11: [norm_full(0, 0)],
                },
            )
            dr()
            ct_h = [dict() for _ in range(TT)]
            # pair 3 (p1,tb1)
            dr = attn_pair(
                1,
                1,
                {},
                {
                    3: [norm_full(1, 0)],
                    5: [c_tile(0)],
                    9: [c_tile(1)],
                    14: [c_tile(2)],
                    19: [c_tile(3)],
                },
            )
            dr()
            # pair 4 (p0,tb1)
            dr = attn_pair(
                0,
                1,
                {},
                {
                    3: [norm_full(1, 1)],
                    5: [c_tile(4)],
                    9: [c_tile(5)],
                    14: [c_tile(6)],
                    19: [c_tile(7)],
                },
            )
            dr()
            norm_half(0, 1, 0)()
            norm_half(0, 1, 1)()
            # tail t-tiles alternate between the qk and uo PSUM slot pools so
            # consecutive tiles don't serialize on slot reuse.
            for j, tt in enumerate(range(TT // 2, TT)):
                pool = (ups, "uo") if j % 2 else (qps, "qk")
                c_tile(tt, on_act=True, pool=pool)()


def _build_nc():
    nc = bacc.Bacc("TRN2", target_bir_lowering=False, debug=False, num_devices=NCORES)
    xT = nc.dram_tensor("xT", [D, T], BF16, kind="ExternalInput").ap()
    ctxT = nc.dram_tensor("ctxT", [D, C], BF16, kind="ExternalInput").ap()
    wqkvT = nc.dram_tensor("wqkvT", [D, 3 * DL], BF16, kind="ExternalInput").ap()
    woT = nc.dram_tensor("woT", [DL, D], BF16, kind="ExternalInput").ap()
    bqkv = nc.dram_tensor("bqkv", [DL, 3], F32, kind="ExternalInput").ap()
    msk = nc.dram_tensor("msk", [65, 128], BF16, kind="ExternalInput").ap()
    iden = nc.dram_tensor("iden", [128, 128], BF16, kind="ExternalInput").ap()
    out = nc.dram_tensor("out", [T, D], BF16, kind="ExternalOutput").ap()
    with tile.TileContext(nc) as tc:
        _emit(nc, tc, (xT, ctxT, wqkvT, woT, bqkv, msk, iden, out))
    nc.compile()
    return nc


_NC_CACHE = None


def _get_nc():
    global _NC_CACHE
    if _NC_CACHE is None:
        _NC_CACHE = _build_nc()
    return _NC_CACHE


def _make_in_maps(inputs):
    x = np.asarray(inputs["x"], dtype=np.float32)
    context = np.asarray(inputs["context"], dtype=np.float32)
    Wq = np.asarray(inputs["Wq"], dtype=np.float32)
    Wk = np.asarray(inputs["Wk"], dtype=np.float32)
    Wv = np.asarray(inputs["Wv"], dtype=np.float32)
    Wo = np.asarray(inputs["Wo"], dtype=np.float32)
    bq = np.asarray(inputs["bq"], dtype=np.float32)
    bk = np.asarray(inputs["bk"], dtype=np.float32)
    bv = np.asarray(inputs["bv"], dtype=np.float32)

    msk = np.zeros((65, 128), _bf16)
    msk[0, :64] = 1.0
    msk[64, 64:] = 1.0
    iden = np.eye(128, dtype=_bf16)

    xTs = [np.ascontiguousarray(x[b].T).astype(_bf16) for b in range(B)]
    cTs = [np.ascontiguousarray(context[b].T).astype(_bf16) for b in range(B)]

    in_maps = []
    for core in range(NCORES):
        b, hg = core // 4, core % 4
        sl = slice(hg * DL, (hg + 1) * DL)
        in_maps.append(
            {
                "xT": xTs[b],
                "ctxT": cTs[b],
                "wqkvT": np.ascontiguousarray(
                    np.concatenate([Wq[sl].T, Wk[sl].T, Wv[sl].T], axis=1)
                ).astype(_bf16),
                "woT": np.ascontiguousarray(Wo[:, sl].T).astype(_bf16),
                "bqkv": np.ascontiguousarray(
                    np.stack([bq[sl], bk[sl], bv[sl]], axis=1)
                ),
                "msk": msk,
                "iden": iden,
            }
        )
    return in_maps


def run_spmd(inputs, trace=False):
    """Run the SPMD kernel; returns (full output [B,T,D], BassKernelResults)."""
    in_maps = _make_in_maps(inputs)
    res = run_bass_kernel_spmd(
        _get_nc(), in_maps, core_ids=list(range(NCORES)), trace=trace
    )
    bo = np.asarray(inputs["bo"], dtype=np.float32)
    y = np.zeros((B, T, D), np.float32)
    for core in range(NCORES):
        y[core // 4] += np.asarray(res.results[core]["out"], dtype=np.float32)
    y += bo.reshape(1, 1, D)
    return y, res


def kernel(**inputs):
    y, _ = run_spmd(inputs, trace=False)
    return y


# revision 32
# speedup vs baseline: 1.0068x; 1.0068x over previous
"""Cross-attention layer (B=2, T=2048, C=3072, D=1024, 16 heads x 64) on 8 TRN2 cores.

Sharding: batch (2) x head-groups (4). Core i handles batch b=i//4 and the 4
heads [4*(i%4), 4*(i%4)+4). Q/K/V projections are column-sharded by head,
the output projection is row-sharded, so each core returns a partial [T, D]
output (bf16); the host sums the 4 partials per batch element and adds bo.

On-core dataflow (bf16 matmuls, fp32 PSUM accum), ~340us/core measured:
  DMA (sync HWDGE queue, priority order): one jumbo descriptor for all
  QKV weights, then x chunks, then ctx chunks; small constants ride the
  scalar HWDGE queue concurrently.
  phase A: Q projection (k-outer over x chunks as they land), then two
  V passes that share one 8-slot PSUM ring with Q and with four early
  K-projection blocks (kT[0] cq0-3) that ride the ctx-DMA-paced PE idle;
  V output is PE-transposed into v[c, (ci,h)*65] (64 v cols + a ones col
  per head from a memset).
  attention runs per head-PAIR (2p, 2p+1), per T-half tb, 24 c-chunks:
  per ci and head: QK (row-tiled at PE position (64*h2, 0), AABB order so
  h0's matmuls are gated only by the early exp slot release), exp on ACT
  (scale=1/8 fused), then the previous ci's PV for that head fills the PE
  under the ACT slot. PV accumulates uo_ps[65, t] (row 64 = softmax
  denominator via the ones column — a free 65th output partition).
  The steady-state ci loop is ACT-bound (2 x 1147ns exp per ci); the
  remaining K blocks (pairs 1-2), denominators/normalization, and the
  first half of the output projection (pairs 3-4) run as just-in-time
  fillers. Every filler allocates an EVEN number of tag-"qk" PSUM tiles
  so the 2-slot ring keeps qk_h0 on the early-released slot.
  normalize: denominators broadcast via mask-matmul, reciprocal+mul on DVE
  out[t, D] = sum_p uo_p^T-slice @ WoT_shard    (bf16, accumulated in PSUM)
"""
from collections import deque

import numpy as np
import concourse.bass as bass  # noqa: F401  (engine types re-exported via nc)
import concourse.mybir as mybir
import concourse.tile as tile
from concourse import bacc
from concourse.bass import ds, ts
from concourse.bass_utils import run_bass_kernel_spmd
import ml_dtypes

_bf16 = ml_dtypes.bfloat16

B, T, C, D = 2, 2048, 3072, 1024
NH, DH = 16, 64
NCORES = 8
HL = 4                # heads per core
DL = HL * DH          # 256 local projection dims
DHP = DH + 1          # 65: per-head v columns incl. ones column

F32 = mybir.dt.float32
BF16 = mybir.dt.bfloat16
AF = mybir.ActivationFunctionType

KC = D // 128         # 8 contraction chunks for projections
CC = C // 128         # 24 key tiles
TT = T // 128         # 16 query tiles


def _emit(nc, tc, io):
    xT, ctxT, wqkvT, woT, bqkv, msk, iden, out = io

    with (
        tc.sbuf_pool(name="persist", bufs=1) as pp,
        tc.sbuf_pool(name="wqkv", bufs=1) as wp,
        tc.sbuf_pool(name="stream", bufs=1) as sp,
    ):
        qT = [pp.tile([128, T], BF16, name=f"qT{p}") for p in range(2)]
        kT = [pp.tile([128, C], BF16, name=f"kT{p}") for p in range(2)]
        v = pp.tile([128, CC * HL * DHP], BF16, name="v")
        wo = [pp.tile([128, D], BF16, name=f"wo{p}") for p in range(2)]
        msk_sb = pp.tile([65, 128], BF16, name="msk_sb")
        iden_sb = pp.tile([128, 128], BF16, name="iden_sb")
        b_sb = [pp.tile([128, 3], F32, name=f"b{p}") for p in range(2)]
        # ones columns for the softmax denominators (65th v column per head):
        # rides the PV matmul as a 65th output partition at zero stream cost.
        ones_view = v.rearrange("a (i c) -> a i c", c=DHP)[:, :, DH:DHP]
        nc.gpsimd.memset(ones_view, 1.0)

        # All weight chunks ride ONE jumbo DMA descriptor into a single
        # [128, KC*768] tile (k-chunk k at free offset k*768): one trigger
        # instead of 8 on the head-critical sync queue.
        w_all = wp.tile([128, KC * 3 * DL], BF16, name="w_all")
        w_sb = [w_all[:, ds(k * 3 * DL, 3 * DL)] for k in range(KC)]
        xs = [sp.tile([128, T], BF16, tag=f"x{k}", name=f"xs{k}") for k in range(KC)]
        cs = [sp.tile([128, C], BF16, tag=f"c{k}", name=f"cs{k}") for k in range(KC)]

        # Priority order on the sync HWDGE queue: w (one jumbo trigger) then
        # x then ctx — everything the projections block on, in need-order so
        # transfers don't compete with each other for HBM bandwidth. The
        # scalar HWDGE queue carries only the small constants concurrently.
        nc.sync.dma_start(
            out=w_all.rearrange("p (k c) -> p k c", k=KC),
            in_=wqkvT.rearrange("(k p) c -> p k c", k=KC),
        )
        for k in range(KC):
            nc.sync.dma_start(out=xs[k], in_=xT[ts(k, 128), :])
        for k in range(KC):
            nc.sync.dma_start(out=cs[k], in_=ctxT[ts(k, 128), :])
        nc.scalar.dma_start(out=iden_sb, in_=iden)
        for p in range(2):
            nc.scalar.dma_start(out=b_sb[p], in_=bqkv[ts(p, 128), :])
        nc.scalar.dma_start(out=msk_sb, in_=msk)
        for p in range(2):
            nc.scalar.dma_start(out=wo[p], in_=woT[ts(p, 128), :])

        bq_sb = [b_sb[p][:, 0:1] for p in range(2)]
        bk_sb = [b_sb[p][:, 1:2] for p in range(2)]
        bv_sb = [b_sb[p][:, 2:3] for p in range(2)]

        wq = [w_sb[k][:, 0:DL] for k in range(KC)]
        wk = [w_sb[k][:, DL : 2 * DL] for k in range(KC)]
        wv = [w_sb[k][:, 2 * DL : 3 * DL] for k in range(KC)]

        # ---------------- Phase A: Q, V, and half of K ----------------
        # One 8-slot PSUM ring ("proj") serves Q, V, early-K, and the
        # transpose staging: V's k-outer groups evict Q's as each Q group
        # drains, and the two K blocks ride the DMA-paced window of each
        # V pass (the PE idles ~0.5us per k-chunk waiting for ctx DMA
        # otherwise). kT[0] cq0-3 is therefore ready before attention
        # starts and pair 1 needs no pre-fillers.
        with (
            tc.sbuf_pool(name="vstage", bufs=6) as vp,
            tc.psum_pool(name="papool", bufs=8) as pa,
        ):
            qgr = [
                pa.tile([128, 512], F32, tag="proj", name=f"qp{p}{tq}")
                for p in range(2)
                for tq in range(4)
            ]
            for k in range(KC):
                for i in range(8):
                    p, tq = i // 4, i % 4
                    nc.tensor.matmul(
                        qgr[i],
                        lhsT=wq[k][:, ts(p, 128)],
                        rhs=xs[k][:, ts(tq, 512)],
                        start=(k == 0),
                        stop=(k == KC - 1),
                    )
            for i in range(8):
                p, tq = i // 4, i % 4
                nc.vector.tensor_scalar_add(
                    out=qT[p][:, ts(tq, 512)], in0=qgr[i], scalar1=bq_sb[p]
                )

            for p in range(2):
                vgr = [
                    pa.tile([128, 512], F32, tag="proj", name=f"vp{p}{cq}")
                    for cq in range(6)
                ]
                kgr = [
                    pa.tile([128, 512], F32, tag="proj", name=f"kg{p}{j}")
                    for j in range(2)
                ]
                for k in range(KC):
                    for cq in range(6):
                        nc.tensor.matmul(
                            vgr[cq],
                            lhsT=wv[k][:, ts(p, 128)],
                            rhs=cs[k][:, ts(cq, 512)],
                            start=(k == 0),
                            stop=(k == KC - 1),
                        )
                    for j in range(2):
                        nc.tensor.matmul(
                            kgr[j],
                            lhsT=wk[k][:, ts(0, 128)],
                            rhs=cs[k][:, ts(2 * p + j, 512)],
                            start=(k == 0),
                            stop=(k == KC - 1),
                        )
                for j in range(2):
                    nc.vector.tensor_scalar_add(
                        out=kT[0][:, ts(2 * p + j, 512)], in0=kgr[j], scalar1=bk_sb[0]
                    )
                vsts = []
                for cq in range(6):
                    vst = vp.tile([128, 512], BF16, tag="vst", name=f"vs{p}{cq}")
                    nc.vector.tensor_scalar_add(out=vst, in0=vgr[cq], scalar1=bv_sb[p])
                    vsts.append(vst)
                for cq in range(6):
                    for cb in range(4):
                        ci = cq * 4 + cb
                        tp_ = pa.tile(
                            [128, 128], BF16, tag="proj", name=f"tr{ci}{p}"
                        )
                        nc.tensor.transpose(tp_, vsts[cq][:, ts(cb, 128)], iden_sb)
                        src_ = tp_.rearrange("a (h c) -> a h c", h=2)
                        dst = v[
                            :, ds(ci * HL * DHP + 2 * p * DHP, 2 * DHP)
                        ].rearrange("a (h c) -> a h c", h=2)[:, :, 0:DH]
                        nc.vector.tensor_copy(out=dst, in_=src_)

        # ---------------- Phases B + C (K-proj + out-proj interleaved) ----
        with (
            tc.sbuf_pool(name="uop", bufs=1) as up,
            tc.sbuf_pool(name="rsp", bufs=1) as rp,
            tc.sbuf_pool(name="obp", bufs=3) as ob,
            tc.sbuf_pool(name="expp", bufs=4) as ep,
            tc.sbuf_pool(name="rcp", bufs=2) as rc,
            tc.psum_pool(name="qkps", bufs=2) as qps,
            tc.psum_pool(name="uops", bufs=2) as ups,
        ):
            uo = [up.tile([128, T], BF16, name=f"uo{pr}") for pr in range(2)]
            rs = [rp.tile([65, T], BF16, name=f"rs{pr}") for pr in range(2)]
            for pr in range(2):
                nc.gpsimd.memset(rs[pr], 1.0)

            def kproj_block(pp_, cq):
                def emit():
                    ps = qps.tile([128, 512], F32, tag="qk", name=f"kp{pp_}{cq}")
                    for k in range(KC):
                        nc.tensor.matmul(
                            ps,
                            lhsT=wk[k][:, ts(pp_, 128)],
                            rhs=cs[k][:, ts(cq, 512)],
                            start=(k == 0),
                            stop=(k == KC - 1),
                        )
                    nc.vector.tensor_scalar_add(
                        out=kT[pp_][:, ts(cq, 512)], in0=ps, scalar1=bk_sb[pp_]
                    )

                return emit

            def kproj_half(pp_, cq):
                a = kproj_quarter(pp_, cq, 0)
                b = kproj_quarter(pp_, cq, 1)

                def emit():
                    a()
                    b()

                return emit

            def kproj_quarter(pp_, cq, qh):
                # 256-wide K-projection block: ~1.3us of PE per invocation so
                # a single filler stays within the per-ci PE headroom under
                # the ACT exp slot (full 512 blocks overshoot and stall ACT).
                def emit():
                    off = cq * 512 + qh * 256
                    ps = qps.tile([128, 256], F32, tag="qk", name=f"kq{pp_}{cq}{qh}")
                    for k in range(KC):
                        nc.tensor.matmul(
                            ps,
                            lhsT=wk[k][:, ts(pp_, 128)],
                            rhs=cs[k][:, ds(off, 256)],
                            start=(k == 0),
                            stop=(k == KC - 1),
                        )
                    nc.vector.tensor_scalar_add(
                        out=kT[pp_][:, ds(off, 256)], in0=ps, scalar1=bk_sb[pp_]
                    )

                return emit

            def c_tile_half(tt, dc, osb_holder, on_act=False, pool=None):
                # half an out-projection t-tile (one 512-wide dc chunk);
                # split so a single filler invocation stays under the per-ci
                # PE headroom while ACT streams the exp.
                def emit():
                    if dc == 0:
                        osb_holder["t"] = ob.tile(
                            [128, D], BF16, tag="ob", name=f"ob{tt}"
                        )
                    osb = osb_holder["t"]
                    pl, tg = pool if pool is not None else (qps, "qk")
                    o_ps = pl.tile([128, 512], F32, tag=tg, name=f"o{tt}_{dc}")
                    for p in range(2):
                        nc.tensor.matmul(
                            o_ps,
                            lhsT=uo[p][:, ts(tt, 128)],
                            rhs=wo[p][:, ts(dc, 512)],
                            start=(p == 0),
                            stop=(p == 1),
                        )
                    if on_act and dc == 1:
                        nc.scalar.copy(out=osb[:, ts(dc, 512)], in_=o_ps)
                    else:
                        nc.vector.tensor_copy(out=osb[:, ts(dc, 512)], in_=o_ps)
                    if dc == 1:
                        nc.sync.dma_start(out=out[ts(tt, 128), :], in_=osb)

                return emit

            def c_tile(tt, on_act=False, pool=None):
                h = {}
                a = c_tile_half(tt, 0, h, on_act, pool)
                b = c_tile_half(tt, 1, h, on_act, pool)

                def emit():
                    a()
                    b()

                return emit

            def norm_full(pr, th):
                a = norm_half(pr, th, 0)
                b = norm_half(pr, th, 1)

                def emit():
                    a()
                    b()

                return emit

            def norm_half(pr, th, tq):
                # broadcast denominators -> [128, 512], recip, scale uo half
                def emit():
                    off = th * 1024 + tq * 512
                    bc = qps.tile([128, 512], F32, tag="qk", name=f"bc{pr}{th}{tq}")
                    nc.tensor.matmul(
                        bc,
                        lhsT=msk_sb,
                        rhs=rs[pr][:, ds(off, 512)],
                        start=True,
                        stop=True,
                    )
                    rcl = rc.tile([128, 512], F32, tag="rc", name=f"rcl{pr}{th}{tq}")
                    nc.vector.reciprocal_approx_fast(rcl, bc)
                    nc.vector.tensor_mul(
                        out=uo[pr][:, ds(off, 512)],
                        in0=uo[pr][:, ds(off, 512)],
                        in1=rcl,
                    )

                return emit

            def attn_pair(p, tb, pre, post):
                # heads (2p, 2p+1) together. QK is row-tiled at PE positions
                # (0,0)/(64,0), AABB per head so h0's matmuls are gated only
                # by exp(ci-1,h0)'s early slot release. The emission is
                # software-pipelined per head: QK_h(ci) and exp_h(ci) issue
                # first, then PV_h(ci-1) — whose ex operand became ready one
                # ACT slot ago — fills the PE while ACT streams the exps, and
                # QK_h1(ci) lands right as exp(ci-1,h1)'s slot frees. PV is
                # the 65-wide form: the ones column in v makes the softmax
                # denominator a free 65th output partition.
                uo_ps = [
                    ups.tile([65, 1024], F32, tag="uo", name=f"up{p}{tb}_{h2}")
                    for h2 in range(2)
                ]

                def pv_step(ci, h2, ex):
                    h = 2 * p + h2
                    vsl = v[:, ds((ci * HL + h) * DHP, DHP)]
                    for tq in range(2):
                        nc.tensor.matmul(
                            uo_ps[h2][:, ts(tq, 512)],
                            lhsT=vsl,
                            rhs=ex[:, ts(tq, 512)],
                            start=(ci == 0),
                            stop=(ci == CC - 1),
                        )

                prev = [None, None]
                for ci in range(CC):
                    for f in pre.get(ci, ()):
                        f()
                    exs = []
                    for h2 in range(2):
                        base = 64 * h2
                        qk = qps.tile(
                            [128, 1024], F32, tag="qk", name=f"qk{p}{tb}_{ci}_{h2}"
                        )
                        for tq in range(2):
                            nc.tensor.matmul(
                                qk[:, ts(tq, 512)],
                                lhsT=kT[p][ds(base, 64), ts(ci, 128)],
                                rhs=qT[p][ds(base, 64), ds(tb * 1024 + tq * 512, 512)],
                                start=True,
                                stop=True,
                                tile_position=(base, 0),
                            )
                        ex = ep.tile(
                            [128, 1024], BF16, tag="exp", name=f"ex{p}{tb}_{ci}_{h2}"
                        )
                        nc.scalar.activation(ex, qk, AF.Exp, scale=0.125)
                        exs.append(ex)
                        if prev[h2] is not None:
                            pv_step(ci - 1, h2, prev[h2])
                    prev = exs
                    for f in post.get(ci, ()):
                        f()
                # tail: last ci's PVs; drain runs on DVE at the boundary.
                for h2 in range(2):
                    pv_step(CC - 1, h2, prev[h2])

                def drain():
                    for h2 in range(2):
                        nc.vector.tensor_copy(
                            out=rs[p][ds(64 * h2, 1), ds(tb * 1024, 1024)],
                            in_=uo_ps[h2][64:65, :],
                        )
                    for h2 in range(2):
                        nc.vector.tensor_copy(
                            out=uo[p][ds(64 * h2, 64), ds(tb * 1024, 1024)],
                            in_=uo_ps[h2][0:64, :],
                        )

                return drain

            # Each pair's drain is emitted at the pair boundary (DVE, off
            # the ACT critical path); norms run as fillers inside the NEXT
            # pair. kT[0] cq0-3 came out of phase A; the remaining K blocks
            # feed pairs 1-2 just-in-time. Every filler allocates an EVEN
            # number of tag-"qk" PSUM tiles so the 2-slot ring's parity is
            # preserved (an odd filler would land the early-gated qk_h0 on
            # the late-released slot and stall the exp stream).
            # pair 1 (p0,tb0)
            dr = attn_pair(
                0,
                0,
                {},
                {
                    1: [kproj_half(0, 4)],
                    5: [kproj_half(0, 5)],
                    9: [kproj_half(1, 0)],
                    13: [kproj_half(1, 1)],
                    17: [kproj_half(1, 2)],
                    21: [kproj_half(1, 3)],
                },
            )
            dr()
            # pair 2 (p1,tb0)
            dr = attn_pair(
                1,
                0,
                {},
                {
                    1: [kproj_half(1, 4)],
                    5: [kproj_half(1, 5)],
                    9: [norm_full(0, 0)],
                },
            )
            dr()
            ct_h = [dict() for _ in range(TT)]
            # pair 3 (p1,tb1)
            dr = attn_pair(
                1,
                1,
                {},
                {
                    2: [norm_full(1, 0)],
                    4: [c_tile(0)],
                    9: [c_tile(1)],
                    14: [c_tile(2)],
                    19: [c_tile(3)],
                },
            )
            dr()
            # pair 4 (p0,tb1)
            dr = attn_pair(
                0,
                1,
                {},
                {
                    2: [norm_full(1, 1)],
                    4: [c_tile(4)],
                    9: [c_tile(5)],
                    14: [c_tile(6)],
                    19: [c_tile(7)],
                },
            )
            dr()
            norm_half(0, 1, 0)()
            norm_half(0, 1, 1)()
            # tail t-tiles alternate between the qk and uo PSUM slot pools so
            # consecutive tiles don't serialize on slot reuse.
            for j, tt in enumerate(range(TT // 2, TT)):
                pool = (ups, "uo") if j % 2 else (qps, "qk")
                c_tile(tt, on_act=True, pool=pool)()


def _build_nc():
    nc = bacc.Bacc("TRN2", target_bir_lowering=False, debug=False, num_devices=NCORES)
    xT = nc.dram_tensor("xT", [D, T], BF16, kind="ExternalInput").ap()
    ctxT = nc.dram_tensor("ctxT", [D, C], BF16, kind="ExternalInput").ap()
    wqkvT = nc.dram_tensor("wqkvT", [D, 3 * DL], BF16, kind="ExternalInput").ap()
    woT = nc.dram_tensor("woT", [DL, D], BF16, kind="ExternalInput").ap()
    bqkv = nc.dram_tensor("bqkv", [DL, 3], F32, kind="ExternalInput").ap()
    msk = nc.dram_tensor("msk", [65, 128], BF16, kind="ExternalInput").ap()
    iden = nc.dram_tensor("iden", [128, 128], BF16, kind="ExternalInput").ap()
    out = nc.dram_tensor("out", [T, D], BF16, kind="ExternalOutput").ap()
    with tile.TileContext(nc) as tc:
        _emit(nc, tc, (xT, ctxT, wqkvT, woT, bqkv, msk, iden, out))
    nc.compile()
    return nc


_NC_CACHE = None


def _get_nc():
    global _NC_CACHE
    if _NC_CACHE is None:
        _NC_CACHE = _build_nc()
    return _NC_CACHE


def _make_in_maps(inputs):
    x = np.asarray(inputs["x"], dtype=np.float32)
    context = np.asarray(inputs["context"], dtype=np.float32)
    Wq = np.asarray(inputs["Wq"], dtype=np.float32)
    Wk = np.asarray(inputs["Wk"], dtype=np.float32)
    Wv = np.asarray(inputs["Wv"], dtype=np.float32)
    Wo = np.asarray(inputs["Wo"], dtype=np.float32)
    bq = np.asarray(inputs["bq"], dtype=np.float32)
    bk = np.asarray(inputs["bk"], dtype=np.float32)
    bv = np.asarray(inputs["bv"], dtype=np.float32)

    msk = np.zeros((65, 128), _bf16)
    msk[0, :64] = 1.0
    msk[64, 64:] = 1.0
    iden = np.eye(128, dtype=_bf16)

    xTs = [np.ascontiguousarray(x[b].T).astype(_bf16) for b in range(B)]
    cTs = [np.ascontiguousarray(context[b].T).astype(_bf16) for b in range(B)]

    in_maps = []
    for core in range(NCORES):
        b, hg = core // 4, core % 4
        sl = slice(hg * DL, (hg + 1) * DL)
        in_maps.append(
            {
                "xT": xTs[b],
                "ctxT": cTs[b],
                "wqkvT": np.ascontiguousarray(
                    np.concatenate([Wq[sl].T, Wk[sl].T, Wv[sl].T], axis=1)
                ).astype(_bf16),
                "woT": np.ascontiguousarray(Wo[:, sl].T).astype(_bf16),
                "bqkv": np.ascontiguousarray(
                    np.stack([bq[sl], bk[sl], bv[sl]], axis=1)
                ),
                "msk": msk,
                "iden": iden,
            }
        )
    return in_maps


def run_spmd(inputs, trace=False):
    """Run the SPMD kernel; returns (full output [B,T,D], BassKernelResults)."""
    in_maps = _make_in_maps(inputs)
    res = run_bass_kernel_spmd(
        _get_nc(), in_maps, core_ids=list(range(NCORES)), trace=trace
    )
    bo = np.asarray(inputs["bo"], dtype=np.float32)
    y = np.zeros((B, T, D), np.float32)
    for core in range(NCORES):
        y[core // 4] += np.asarray(res.results[core]["out"], dtype=np.float32)
    y += bo.reshape(1, 1, D)
    return y, res


def kernel(**inputs):
    y, _ = run_spmd(inputs, trace=False)
    return y
